# revision 49
# baseline (speedup 1.0000x reference)
"""Trainium2 Bass kernel for nn_CoreBlock (circulant attention + 2-layer FFN).

Contract: kernel(**inputs) takes FULL unsharded inputs (as produced by
setup_inputs) and returns the FULL [16, 1024, 768] f32 output.

Strategy: pure data-parallel over batch - 8 NeuronCores x 2 batches each.
All weights replicated. Per core (restructured for engine balance):

  phase A (per 4-chunk group): batched input DMA, bn_stats (DVE),
     rstd = recip(ACT Sqrt(var+eps)) (exact, 2 tiny ops), LayerNorm as a
     single ACT Copy(scale=rstd, bias=-mu*rstd), ONE batched XBAR
     DMA-transpose of the 4-chunk u tile, then v-projection matmuls.
     No PE transposes, no DVE normalize pass.
  phase B (per batch): circulant matmul with the 8-tile Toeplitz bank,
     free dim = jc-run * HS (<=256). Residual-added into X.
  phase C (per batch): 2x [Dense -> LayerNorm -> swish]. Row-sums of the
     dense output come free from a colsum column appended to Wf (PSUM col
     768); ssq via one DVE accum op; rstd via bit-hack + 1 Newton step
     (batched per group, all tiny DVE ops); Silu with scale/bias fused.
     Each 4-chunk group uses ONE batched XBAR DMA-transpose.
  tail (per batch): log_cosh(w) = |w| + log1p(exp(-2|w|)) - ln2 with the
     Exp/Ln table fence; outputs DMA'd per 4-chunk block on the gpsimd
     software queue.

Emission order A, B0, C0, B1, tail0, C1, tail1 lets batch-0's scalar/
vector tail run under batch-1's PE phases, so the PE never waits long
and the kernel ends shortly after the last matmul.

Matmul operands are bf16 (full-rate PE, fp32 PSUM accumulation); stats
and elementwise math fp32.
"""

import math
import numpy as np
import ml_dtypes

import concourse.bass as bass
import concourse.tile as tile
from concourse import bacc, mybir
from concourse.bass_utils import run_bass_kernel_spmd

BF16 = ml_dtypes.bfloat16

B, N, D = 16, 1024, 768
H, HS, L = 12, 64, 2
EPS = 1e-6
NCORES = 8
BPC = B // NCORES          # batches per core
NJ = N // 128              # token chunks per batch (8)
NT = BPC * NJ              # token chunks per core (16)
DC = D // 128              # feature chunks (6)
AB = 4                     # chunks per group
FW = 776                   # wf free width: 768 outputs + colsum + pad

F32 = mybir.dt.float32
I32 = mybir.dt.int32
BF = mybir.dt.bfloat16
Alu = mybir.AluOpType
Act = mybir.ActivationFunctionType

LN2 = math.log(2.0)
# fp32 whose bit pattern is 0x5f3759df (fast-rsqrt magic constant)
MAGIC_F = float(np.int32(0x5F3759DF).view(np.float32))

TRACE = False              # test harness sets this for profiling runs
TRACE_KW = {}
DEBUG = False              # adds intermediate-dump outputs (debugging only)

_cache = {}


class _Bacc(bacc.Bacc):
    """Bacc whose activation-table chooser sees Exp and Ln only in the
    combined natural_log_exp_and_others set, so the log_cosh tail needs one
    table load instead of alternating exp/ln loads."""

    def insert_act_table_loads(self):
        has_activation = any(
            isinstance(i, mybir.InstActivation)
            for b in self.main_func.blocks
            for i in b.instructions
        )
        if not has_activation:
            return
        from concourse.hw_specs import get_activation_tables
        Fn = mybir.ActivationFunctionType
        tables = []
        for name, fns in get_activation_tables(self.m.arch).items():
            if name != "natural_log_exp_and_others":
                fns = fns - {Fn.Exp, Fn.Ln}
            tables.append((name, fns))
        import concourse._compat as _compat  # noqa: F401
        from concourse.bacc import _bass_rust
        _bass_rust.insert_act_table_loads(self, tables)


def _build(cv_nonzero, bf_nonzero, lnf_uniform):
    nc = _Bacc("TRN2", target_bir_lowering=False, debug=False)

    # weights arrive pre-arranged as [partition, ...contiguous] so each
    # DMA is one large descriptor per partition
    xs = nc.dram_tensor("xs", (BPC, N, D), F32, kind="ExternalInput").ap()
    wv = nc.dram_tensor("wv", (128, DC, D), BF, kind="ExternalInput").ap()
    wf = nc.dram_tensor("wf", (128, L, DC, FW), BF, kind="ExternalInput").ap()
    tb_d = nc.dram_tensor("tbank", (128, H, NJ, 128), BF, kind="ExternalInput").ap()
    cv_d = nc.dram_tensor("cv", (D,), F32, kind="ExternalInput").ap()
    bf_d = nc.dram_tensor("bfb", (L, D), F32, kind="ExternalInput").ap()
    lnfs_d = nc.dram_tensor("lnfs", (L, D), F32, kind="ExternalInput").ap()
    lnfb_d = nc.dram_tensor("lnfb", (L, D), F32, kind="ExternalInput").ap()
    out_d = nc.dram_tensor("out", (BPC, N, D), F32, kind="ExternalOutput").ap()

    dbg = None
    if DEBUG:
        dbg = {
            "dbg_u": nc.dram_tensor("dbg_u", (128, AB, D), BF,
                                    kind="ExternalOutput").ap(),
            "dbg_v": nc.dram_tensor("dbg_v", (128, H, NJ, BPC, HS), BF,
                                    kind="ExternalOutput").ap(),
            "dbg_x1": nc.dram_tensor("dbg_x1", (128, BPC, NJ, D), F32,
                                     kind="ExternalOutput").ap(),
            "dbg_y2": nc.dram_tensor("dbg_y2", (128, AB, D), BF,
                                     kind="ExternalOutput").ap(),
            "dbg_z2": nc.dram_tensor("dbg_z2", (128, NJ, D), BF,
                                     kind="ExternalOutput").ap(),
        }

    with tile.TileContext(nc) as tc:
        _emit(nc, tc, xs, wv, wf, tb_d, cv_d, bf_d, lnfs_d, lnfb_d,
              out_d, cv_nonzero, bf_nonzero, lnf_uniform, dbg)
    nc.compile()
    return nc


def _newton1_rstd(nc, pool, dst, var_ap, magict, g):
    """dst[128, g] = 1/sqrt(var + EPS) via bit-hack seed + 1 Newton step
    (max rel err ~0.17%). All tiny DVE ops."""
    vv = pool.tile([128, 16], F32, tag="nvv", name="nvv")[:, 0:g]
    nc.vector.tensor_scalar(vv, var_ap, EPS, None, op0=Alu.add)
    y0 = pool.tile([128, 16], F32, tag="ny0", name="ny0")[:, 0:g]
    nc.vector.tensor_scalar(y0.bitcast(I32), vv.bitcast(I32), 1, None,
                            op0=Alu.logical_shift_right)
    nc.vector.tensor_tensor(y0.bitcast(I32), magict[:, 0:g].bitcast(I32),
                            y0.bitcast(I32), op=Alu.subtract)
    t1 = pool.tile([128, 16], F32, tag="nt1", name="nt1")[:, 0:g]
    nc.vector.tensor_tensor(t1, y0, y0, op=Alu.mult)
    nc.vector.tensor_tensor(t1, t1, vv, op=Alu.mult)
    nc.vector.tensor_scalar(t1, t1, -0.5, 1.5, op0=Alu.mult, op1=Alu.add)
    nc.vector.tensor_tensor(dst, y0, t1, op=Alu.mult)


def _emit(nc, tc, xs, wv, wf, tb_d, cv_d, bf_d, lnfs_d, lnfb_d,
          out_d, cv_nonzero, bf_nonzero, lnf_uniform, dbg=None):
    from contextlib import ExitStack
    ctx = ExitStack()
    with ctx:
        consts = ctx.enter_context(tc.tile_pool(name="consts", bufs=1))
        wp_tb = ctx.enter_context(tc.tile_pool(name="wp_tb", bufs=1))
        wp_wf = ctx.enter_context(tc.tile_pool(name="wp_wf", bufs=1))
        xpool = ctx.enter_context(tc.tile_pool(name="xpool", bufs=1))
        vpool = ctx.enter_context(tc.tile_pool(name="vpool", bufs=1))
        z2p = ctx.enter_context(tc.tile_pool(name="z2p", bufs=2))
        srcp = ctx.enter_context(tc.tile_pool(name="srcp", bufs=3))
        dtp = ctx.enter_context(tc.tile_pool(name="dtp", bufs=3))
        y1p = ctx.enter_context(tc.tile_pool(name="y1p", bufs=2))
        scrp = ctx.enter_context(tc.tile_pool(name="scrp", bufs=2))
        stat = ctx.enter_context(tc.tile_pool(name="stat", bufs=4))
        statp = ctx.enter_context(tc.tile_pool(name="statp", bufs=1))
        ps_mm = ctx.enter_context(tc.tile_pool(name="ps_mm", bufs=4, space="PSUM"))

        # ---- constants ----
        wv_s = consts.tile([128, DC, D], BF, tag="wv")
        # tb/wf DMAs are gated (below) so the 6.6MB of weights don't steal
        # HBM bandwidth from the input-x transfers; the gate tiles alias the
        # weight buffers and are read by a dummy op that depends on a late
        # input chunk's stats.
        tb_gate = wp_tb.tile([128, H, NJ, 128], BF, tag="tb", name="tb_gate")
        wf_gate = wp_wf.tile([128, L, DC, FW], BF, tag="wf", name="wf_gate")
        nc.vector.memset(tb_gate[:, 0, 0, 0:1], 0.0)
        nc.vector.memset(wf_gate[:, 0, 0, 0:1], 0.0)
        magict = consts.tile([128, 16], F32, tag="magic")
        nc.vector.memset(magict[:], MAGIC_F)
        onet = consts.tile([128, 1], F32, tag="one")
        nc.vector.memset(onet[:], 1.0)
        zerot = consts.tile([128, 1], F32, tag="zero")
        nc.vector.memset(zerot[:], 0.0)
        epst = consts.tile([128, 1], F32, tag="eps")
        nc.vector.memset(epst[:], EPS)

        # weight DMAs all go through the gpsimd software queue; tb/wf are
        # emitted mid-phase-A (below) so the input-x transfers get HBM
        # bandwidth first.
        nc.gpsimd.dma_start(wv_s[:], wv)

        cvt = None
        if cv_nonzero:
            cvt = consts.tile([128, D], F32, tag="cv")
            nc.gpsimd.dma_start(cvt[:], cv_d.to_broadcast((128, D)))
        bft = [None] * L
        lnfst = [None] * L
        lnfbt = [None] * L
        for l in range(L):
            if bf_nonzero[l]:
                bft[l] = consts.tile([128, D], F32, tag=f"bf{l}")
                nc.gpsimd.dma_start(bft[l][:], bf_d[l].to_broadcast((128, D)))
            if lnf_uniform[l] is None:
                lnfst[l] = consts.tile([128, D], F32, tag=f"lnfs{l}")
                nc.gpsimd.dma_start(lnfst[l][:], lnfs_d[l].to_broadcast((128, D)))
                lnfbt[l] = consts.tile([128, D], F32, tag=f"lnfb{l}")
                nc.gpsimd.dma_start(lnfbt[l][:], lnfb_d[l].to_broadcast((128, D)))

        # ---- resident tensors ----
        X = xpool.tile([128, BPC, NJ, D], F32, tag="X")         # x, then x1
        V = vpool.tile([128, H, NJ, BPC, HS], BF, tag="V")      # per-head values

        weights = {}
        # ================= phase A: LN + v-projection =================
        # LN's rstd commutes through the matmul: project (mu - x) @ Wv
        # (norm = one ACT Identity with bias = raw mean, nothing else on the
        # pre-matmul critical path), then scale V by -rstd in the post-matmul
        # copy. The reciprocal runs off-path on the DVE.
        mvA = statp.tile([128, NT, 2], F32, tag="mvA")
        nrsA = statp.tile([128, NT], F32, tag="nrsA")
        for g in range(NT // AB):
            b, jc0 = divmod(g * AB, NJ)
            # input DMAs per 2 chunks over three queues: few enough that the
            # DMA semaphore slots don't wrap onto the phase-A transposes
            # (WAR guards), spread for aggregate HBM bandwidth
            engs = (nc.sync, nc.scalar, nc.gpsimd)
            for h2 in range(AB // 2):
                jc = jc0 + h2 * 2
                eng = engs[(g * 2 + h2) % 3]
                eng.dma_start(
                    X[:, b, jc:jc + 2, :],
                    xs[b, jc * 128:(jc + 2) * 128, :].rearrange(
                        "(c p) d -> p c d", p=128))
            ug = srcp.tile([128, AB, D], BF, tag="src", name="ug")
            for ti in range(AB):
                t = g * AB + ti
                xt = X[:, b, jc0 + ti, :]
                st = stat.tile([128, 2, 6], F32, tag="bst")
                nc.vector.bn_stats(st[:, 0, :], xt[:, 0:512])
                nc.vector.bn_stats(st[:, 1, :], xt[:, 512:D])
                nc.vector.bn_aggr(mvA[:, t, :], st[:])
                # u~ = mu - x  (no rstd yet; fires right after this chunk's
                # aggr, independent of any other chunk)
                nc.scalar.activation(ug[:, ti, :], xt, Act.Identity,
                                     bias=mvA[:, t, 0:1], scale=-1.0)
            if g == 2:
                gd = stat.tile([128, 1], F32, tag="gd", name="gd_tb")
                nc.vector.scalar_tensor_tensor(
                    gd[:], mvA[:, g * AB + 3, 0:1], 1.0,
                    tb_gate[:, 0, 0, 0:1], op0=Alu.mult, op1=Alu.add)
                tb_s = wp_tb.tile([128, H, NJ, 128], BF, tag="tb")
                nc.gpsimd.dma_start(tb_s[:], tb_d)
                weights["tb"] = tb_s
            if g == 3:
                gd = stat.tile([128, 1], F32, tag="gd", name="gd_wf")
                nc.vector.scalar_tensor_tensor(
                    gd[:], mvA[:, g * AB + 3, 0:1], 1.0,
                    wf_gate[:, 0, 0, 0:1], op0=Alu.mult, op1=Alu.add)
                wf_s = wp_wf.tile([128, L, DC, FW], BF, tag="wf")
                nc.gpsimd.dma_start(wf_s[:], wf)
                weights["wf"] = wf_s
            gs = slice(g * AB, g * AB + AB)
            # -rstd = reciprocal(-sqrt(var+eps)); off the critical path
            stdt = stat.tile([128, 16], F32, tag="astd", name="astd")[:, 0:AB]
            nc.scalar.activation(stdt, mvA[:, gs, 1], Act.Sqrt, bias=epst[:])
            nstdt = stat.tile([128, 16], F32, tag="anstd", name="anstd")[:, 0:AB]
            nc.scalar.activation(nstdt, stdt, Act.Identity, scale=-1.0)
            nc.vector.reciprocal(nrsA[:, gs], nstdt)
            if dbg is not None and g == 0:
                nc.gpsimd.dma_start(dbg["dbg_u"], ug[:])
            udt = dtp.tile([128, AB * DC, 128], BF, tag="dt", name="udt")
            nc.sync.dma_start_transpose(
                udt[:], ug[:].rearrange("p a d -> p (a d)"))
            for ti in range(AB):
                t = g * AB + ti
                jc = jc0 + ti
                pv = ps_mm.tile([128, 1024], F32, tag="mm")
                for c in range(DC):
                    nc.tensor.matmul(pv[:, 0:512], udt[:, ti * DC + c, :],
                                     wv_s[:, c, 0:512],
                                     start=(c == 0), stop=(c == DC - 1))
                    nc.tensor.matmul(pv[:, 512:D], udt[:, ti * DC + c, :],
                                     wv_s[:, c, 512:D],
                                     start=(c == 0), stop=(c == DC - 1))
                pv3 = pv[:, 0:D].rearrange("p (h k) -> p h k", h=H)
                if cv_nonzero:
                    cv3 = cvt[:].rearrange("p (h k) -> p h k", h=H)
                    nc.vector.scalar_tensor_tensor(V[:, :, jc, b, :], pv3,
                                                   nrsA[:, t:t + 1], cv3,
                                                   op0=Alu.mult, op1=Alu.add)
                else:
                    # V = -rstd * pv = rstd * (x - mu) @ Wv
                    nc.scalar.activation(V[:, :, jc, b, :], pv3, Act.Identity,
                                         scale=nrsA[:, t:t + 1])

        if dbg is not None:
            nc.gpsimd.dma_start(dbg["dbg_v"], V[:])

        # stats tiles for phase C
        ssqC = statp.tile([128, L, NT], F32, tag="ssqC")
        muC = statp.tile([128, L, NT], F32, tag="muC")
        rsC = statp.tile([128, L, NT], F32, tag="rsC")
        biasC = statp.tile([128, L, NT], F32, tag="biasC")
        fence = statp.tile([128, 4], F32, tag="fence")
        mvC = statp.tile([128, AB, 2], F32, tag="mvC")
        inv_d = 1.0 / D

        def phase_b(b):
            # y[ic] = sum_m T[m] @ V[(ic+m) % NJ]; residual into X.
            # two heads share one 2-bank PSUM tile (one accumulation bank
            # each) and get a single fused residual add, halving the DVE op
            # count and the PSUM WAR pressure.
            for h0 in range(0, H, 2):
                pc = ps_mm.tile([128, 2, NJ, HS], F32, tag="mm", name="pc")
                for hh in range(2):
                    h = h0 + hh
                    for m in range(NJ):
                        # one MM per run (free = ln*HS <= 512): a PSUM bank
                        # sees exactly one start=True MM
                        for ic0, jc0, ln in ((0, m, NJ - m), (NJ - m, 0, m)):
                            if ln == 0:
                                continue
                            nc.tensor.matmul(
                                pc[:, hh, ic0:ic0 + ln, :],
                                weights["tb"][:, h, m, :],
                                V[:, h, jc0:jc0 + ln, b, :],
                                start=(m == 0), stop=(m == NJ - 1),
                                skip_group_check=True)
                xap = X[:, b, :, h0 * HS:(h0 + 2) * HS]
                nc.vector.tensor_tensor(
                    xap, xap, pc[:].rearrange("p h j k -> p j h k"),
                    op=Alu.add)

        def c_epi(b, l, g, jc0, o, n, fast, y1g, pffs, y2g, Z2h,
                  var_src=None):
            """stats -> rstd -> Silu for chunk slice [o, o+n) of group g."""
            t0 = b * NJ + jc0 + o
            ts = slice(t0, t0 + n)
            if var_src is None:
                m2 = stat.tile([128, 16], F32, tag="m2", name="m2")[:, 0:n]
                nc.vector.tensor_scalar(m2, ssqC[:, l, ts], inv_d, None,
                                        op0=Alu.mult)
                var = stat.tile([128, 16], F32, tag="var", name="var")[:, 0:n]
                nc.vector.scalar_tensor_tensor(var, muC[:, l, ts], -1.0,
                                               muC[:, l, ts], op0=Alu.mult,
                                               op1=Alu.mult)
                nc.vector.tensor_tensor(var, m2, var, op=Alu.add)
            else:
                var = var_src
            _newton1_rstd(nc, stat, rsC[:, l, ts], var, magict, n)
            if fast:
                cs, cb = lnf_uniform[l]
                if cs != 1.0:
                    nc.vector.tensor_scalar(rsC[:, l, ts], rsC[:, l, ts],
                                            float(cs), None, op0=Alu.mult)
                nc.vector.scalar_tensor_tensor(biasC[:, l, ts], muC[:, l, ts],
                                               -1.0, rsC[:, l, ts],
                                               op0=Alu.mult, op1=Alu.mult)
                if cb != 0.0:
                    nc.vector.tensor_scalar(biasC[:, l, ts], biasC[:, l, ts],
                                            float(cb), None, op0=Alu.add)
                for k in range(n):
                    ti = o + k
                    t = t0 + k
                    ysrc = (pffs[ti][:, 0:D] if pffs[ti] is not None
                            else y1g[:, ti, 0:D])
                    dst = y2g[g][:, ti, :] if l == 0 else Z2h[:, jc0 + ti, :]
                    nc.scalar.activation(dst, ysrc, Act.Silu,
                                         bias=biasC[:, l, t:t + 1],
                                         scale=rsC[:, l, t:t + 1])
            else:
                for k in range(n):
                    ti = o + k
                    t = t0 + k
                    tmp = scrp.tile([128, D], BF, tag="scr", name="lnf_tmp")
                    nc.vector.tensor_scalar(tmp[:], y1g[:, ti, 0:D],
                                            muC[:, l, t:t + 1],
                                            rsC[:, l, t:t + 1],
                                            op0=Alu.subtract, op1=Alu.mult)
                    nc.vector.tensor_tensor(tmp[:], tmp[:], lnfst[l][:],
                                            op=Alu.mult)
                    dst = y2g[g][:, ti, :] if l == 0 else Z2h[:, jc0 + ti, :]
                    nc.vector.tensor_tensor(dst, tmp[:], lnfbt[l][:],
                                            op=Alu.add)
                    nc.scalar.activation(dst, dst, Act.Silu, bias=zerot[:])

        def phase_c(b, tail_cb=None):
            y2g = [None, None]
            xbg = []
            # bf16 casts of x1 up front so they land early in the DVE FIFO
            for g in range(NJ // AB):
                xb = srcp.tile([128, AB, D], BF, tag="src", name="xb")
                nc.vector.tensor_copy(xb[:], X[:, b, g * AB:(g + 1) * AB, :])
                xbg.append(xb)
            Z2h = z2p.tile([128, NJ, D], BF, tag="z2", name=f"z2_{b}")
            for l in range(L):
                fast = lnf_uniform[l] is not None
                for g in range(NJ // AB):
                    jc0 = g * AB
                    src = xbg[g] if l == 0 else y2g[g]
                    # (direct-PSUM epilogue measured as a net loss: it holds
                    # PSUM until the Silus and stalls the final matmuls)
                    last = False
                    if l == 0 and y2g[g] is None:
                        y2g[g] = srcp.tile([128, AB, D], BF, tag="src",
                                           name="y2")
                    y1g = y1p.tile([128, AB, FW], BF, tag="y1")
                    pffs = [None] * AB
                    # two transpose waves of 2 chunks each: the first wave's
                    # matmuls start ~2us after the sources are ready
                    for w in range(AB // 2):
                        zdt = dtp.tile([128, 2 * DC, 128], BF, tag="dt",
                                       name="zdt")
                        nc.sync.dma_start_transpose(
                            zdt[:], src[:, w * 2:w * 2 + 2, :].rearrange(
                                "p a d -> p (a d)"))
                        for wi in range(2):
                            ti = w * 2 + wi
                            t = b * NJ + jc0 + ti
                            pff = ps_mm.tile([128, 1024], F32, tag="mm")
                            for c in range(DC):
                                nc.tensor.matmul(pff[:, 0:512],
                                                 zdt[:, wi * DC + c, :],
                                                 weights["wf"][:, l, c, 0:512],
                                                 start=(c == 0),
                                                 stop=(c == DC - 1))
                                nc.tensor.matmul(pff[:, 512:FW],
                                                 zdt[:, wi * DC + c, :],
                                                 weights["wf"][:, l, c, 512:FW],
                                                 start=(c == 0),
                                                 stop=(c == DC - 1))
                            if bf_nonzero[l]:
                                nc.vector.tensor_tensor(pff[:, 0:D],
                                                        pff[:, 0:D],
                                                        bft[l][:], op=Alu.add)
                            if last:
                                pffs[ti] = pff
                                # stats straight from PSUM via bn_stats (a
                                # single-input op, unlike the ssq form)
                                stc = stat.tile([128, 2, 6], F32, tag="bst",
                                                name="bstC")
                                nc.vector.bn_stats(stc[:, 0, :], pff[:, 0:512])
                                nc.vector.bn_stats(stc[:, 1, :], pff[:, 512:D])
                                nc.vector.bn_aggr(mvC[:, ti, :], stc[:])
                            else:
                                # copy 769 cols: dense output + its row-sum
                                nc.scalar.copy(y1g[:, ti, 0:D + 1],
                                               pff[:, 0:D + 1])
                                scr = scrp.tile([128, D], BF, tag="scr")
                                nc.vector.scalar_tensor_tensor(
                                    scr[:], y1g[:, ti, 0:D], 0.0,
                                    y1g[:, ti, 0:D],
                                    op0=Alu.add, op1=Alu.mult,
                                    accum_out=ssqC[:, l, t:t + 1])
                        if last:
                            # per-wave epilogue (2 PSUM tiles live at a time)
                            t0w = b * NJ + jc0 + w * 2
                            nc.vector.tensor_copy(
                                muC[:, l, t0w:t0w + 2], mvC[:, w * 2:w * 2 + 2, 0])
                            c_epi(b, l, g, jc0, w * 2, 2, fast, y1g, pffs,
                                  y2g, Z2h,
                                  var_src=mvC[:, w * 2:w * 2 + 2, 1])
                    if not last:
                        t0 = b * NJ + jc0
                        ts = slice(t0, t0 + AB)
                        nc.vector.tensor_scalar(muC[:, l, ts], y1g[:, :, D],
                                                inv_d, None, op0=Alu.mult)
                        c_epi(b, l, g, jc0, 0, AB, fast, y1g, pffs, y2g, Z2h)
                    if dbg is not None and b == 0 and l == 0 and g == 0:
                        nc.gpsimd.dma_start(dbg["dbg_y2"], y2g[g][:])
                    if l == 1:
                        # the tail's add+abs fold in right behind this
                        # group's Silus (spreads DVE out of the tail window)
                        for ti in range(AB):
                            jc = jc0 + ti
                            xt = X[:, b, jc, :]
                            nc.vector.tensor_tensor(xt, xt, Z2h[:, jc, :],
                                                    op=Alu.add)
                            nc.vector.scalar_tensor_tensor(xt, xt, -1.0, xt,
                                                           op0=Alu.mult,
                                                           op1=Alu.max)
                        if tail_cb is not None:
                            tail_cb(g, Z2h)
            return Z2h

        def tail_part(b, Z2h, jlo, jn, fcell):
            # log_cosh(w) = |w| + log1p(exp(-2|w|)) - ln2; X already holds
            # |w| (add+abs ran inside phase C)
            # fence: depends on the covered layer-2 Silus, used as the Exp
            # bias so tail Exps can't interleave with Silus (table thrash)
            fscr = stat.tile([128, NJ], F32, tag="fscr", name="fscr")[:, 0:jn]
            nc.vector.tensor_scalar(fscr, Z2h[:, jlo:jlo + jn, 0], 0.0, 0.0,
                                    op0=Alu.mult, op1=Alu.mult,
                                    accum_out=fence[:, fcell:fcell + 1])
            for jc in range(jlo, jlo + jn):
                nc.scalar.activation(Z2h[:, jc, :], X[:, b, jc, :], Act.Exp,
                                     bias=fence[:, fcell:fcell + 1],
                                     scale=-2.0)
            for jc in range(jlo, jlo + jn):
                nc.scalar.activation(Z2h[:, jc, :], Z2h[:, jc, :], Act.Ln,
                                     bias=onet[:], scale=1.0)
            for half in range(jn // 2):
                j0 = jlo + half * 2
                for jc in range(j0, j0 + 2):
                    nc.vector.scalar_tensor_tensor(
                        X[:, b, jc, :], Z2h[:, jc, :], -LN2, X[:, b, jc, :],
                        op0=Alu.add, op1=Alu.add)
                nc.sync.dma_start(
                    out_d[b, j0 * 128:(j0 + 2) * 128, :].rearrange(
                        "(c p) d -> p c d", p=128),
                    X[:, b, j0:j0 + 2, :])

        phase_b(0)
        if dbg is not None:
            nc.gpsimd.dma_start(dbg["dbg_x1"][:, 0], X[:, 0])
        Z2_0 = phase_c(0)
        if dbg is not None:
            nc.gpsimd.dma_start(dbg["dbg_z2"], Z2_0[:])
        phase_b(1)
        if dbg is not None:
            nc.gpsimd.dma_start(dbg["dbg_x1"][:, 1], X[:, 1])
        tail_part(0, Z2_0, 0, NJ, 0)
        # batch 1's tail is split per 4-chunk half and emitted inside
        # phase C so the first half overlaps the final matmuls
        Z2_1 = phase_c(1, tail_cb=lambda g, Z: tail_part(1, Z, g * AB, AB,
                                                         2 + g))


def _prep(inputs):
    x = np.asarray(inputs["x"], np.float32)
    ln1_s = np.asarray(inputs["ln1_scale"], np.float32)
    ln1_b = np.asarray(inputs["ln1_bias"], np.float32)
    Wv = np.asarray(inputs["Wv"], np.float32)
    alpha = np.asarray(inputs["alpha"], np.float32)
    Wf = np.asarray(inputs["Wf"], np.float32)
    bfv = np.asarray(inputs["bf"], np.float32)
    lnf_s = np.asarray(inputs["lnf_scale"], np.float32)
    lnf_b = np.asarray(inputs["lnf_bias"], np.float32)

    Wv_flat = Wv.transpose(1, 0, 2).reshape(D, H * HS)
    Wvp = (ln1_s[:, None] * Wv_flat).astype(BF16)
    cv = (ln1_b @ Wv_flat).astype(np.float32)

    # Wf extended with a colsum column (row-sums of the dense output come
    # from the matmul itself) and zero padding to FW
    Wf_ext = np.zeros((L, D, FW), np.float32)
    Wf_ext[:, :, 0:D] = Wf
    Wf_ext[:, :, D] = Wf.sum(axis=2)

    ar = alpha[:, (-np.arange(N)) % N]
    ar2 = np.concatenate([ar, ar], axis=1)
    m_ = np.arange(NJ)[:, None, None]
    p_ = np.arange(128)[None, :, None]
    f_ = np.arange(128)[None, None, :]
    T = ar2[:, N + 128 * m_ + p_ - f_]                  # [H, NJ, 128, 128]
    # [128, H, NJ, 128]: partition-major so the DMA is contiguous
    tbank = np.ascontiguousarray(T.transpose(2, 0, 1, 3)).astype(BF16)

    cv_nonzero = bool(np.any(cv))
    bf_nonzero = tuple(bool(np.any(bfv[l])) for l in range(L))
    lnf_uniform = []
    for l in range(L):
        s, bb = lnf_s[l], lnf_b[l]
        if np.all(s == s[0]) and np.all(bb == bb[0]):
            lnf_uniform.append((float(s[0]), float(bb[0])))
        else:
            lnf_uniform.append(None)
    key = (cv_nonzero, bf_nonzero, tuple(lnf_uniform))

    # partition-major weight layouts for contiguous DMA:
    # wv: [D, H*HS] -> [128, DC, D_out];  wf: [L, D, FW] -> [128, L, DC, FW]
    wv_pm = np.ascontiguousarray(
        Wvp.reshape(DC, 128, D).transpose(1, 0, 2))
    wf_pm = np.ascontiguousarray(
        Wf_ext.astype(BF16).reshape(L, DC, 128, FW).transpose(2, 0, 1, 3))
    common = {
        "wv": wv_pm,
        "wf": wf_pm,
        "tbank": tbank,
        "cv": cv,
        "bfb": bfv,
        "lnfs": lnf_s,
        "lnfb": lnf_b,
    }
    return x, key, common, (cv_nonzero, bf_nonzero, lnf_uniform)


def kernel(**inputs):
    x, key, common, flags = _prep(inputs)
    if key not in _cache:
        _cache[key] = _build(*flags)
    nc = _cache[key]
    in_maps = []
    for i in range(NCORES):
        m = dict(common)
        m["xs"] = np.ascontiguousarray(x[i * BPC:(i + 1) * BPC])
        in_maps.append(m)
    res = run_bass_kernel_spmd(nc, in_maps, core_ids=list(range(NCORES)),
                               trace=TRACE, **TRACE_KW)
    kernel.last_result = res
    out = np.empty((B, N, D), np.float32)
    for i in range(NCORES):
        out[i * BPC:(i + 1) * BPC] = res.results[i]["out"]
    return out


# revision 50
# speedup vs baseline: 1.0024x; 1.0024x over previous
"""Trainium2 Bass kernel for nn_CoreBlock (circulant attention + 2-layer FFN).

Contract: kernel(**inputs) takes FULL unsharded inputs (as produced by
setup_inputs) and returns the FULL [16, 1024, 768] f32 output.

Strategy: pure data-parallel over batch - 8 NeuronCores x 2 batches each.
All weights replicated. Per core (restructured for engine balance):

  phase A (per 4-chunk group): batched input DMA, bn_stats (DVE),
     rstd = recip(ACT Sqrt(var+eps)) (exact, 2 tiny ops), LayerNorm as a
     single ACT Copy(scale=rstd, bias=-mu*rstd), ONE batched XBAR
     DMA-transpose of the 4-chunk u tile, then v-projection matmuls.
     No PE transposes, no DVE normalize pass.
  phase B (per batch): circulant matmul with the 8-tile Toeplitz bank,
     free dim = jc-run * HS (<=256). Residual-added into X.
  phase C (per batch): 2x [Dense -> LayerNorm -> swish]. Row-sums of the
     dense output come free from a colsum column appended to Wf (PSUM col
     768); ssq via one DVE accum op; rstd via bit-hack + 1 Newton step
     (batched per group, all tiny DVE ops); Silu with scale/bias fused.
     Each 4-chunk group uses ONE batched XBAR DMA-transpose.
  tail (per batch): log_cosh(w) = |w| + log1p(exp(-2|w|)) - ln2 with the
     Exp/Ln table fence; outputs DMA'd per 4-chunk block on the gpsimd
     software queue.

Emission order A, B0, C0, B1, tail0, C1, tail1 lets batch-0's scalar/
vector tail run under batch-1's PE phases, so the PE never waits long
and the kernel ends shortly after the last matmul.

Matmul operands are bf16 (full-rate PE, fp32 PSUM accumulation); stats
and elementwise math fp32.
"""

import math
import numpy as np
import ml_dtypes

import concourse.bass as bass
import concourse.tile as tile
from concourse import bacc, mybir
from concourse.bass_utils import run_bass_kernel_spmd

BF16 = ml_dtypes.bfloat16

B, N, D = 16, 1024, 768
H, HS, L = 12, 64, 2
EPS = 1e-6
NCORES = 8
BPC = B // NCORES          # batches per core
NJ = N // 128              # token chunks per batch (8)
NT = BPC * NJ              # token chunks per core (16)
DC = D // 128              # feature chunks (6)
AB = 4                     # chunks per group
FW = 776                   # wf free width: 768 outputs + colsum + pad

F32 = mybir.dt.float32
I32 = mybir.dt.int32
BF = mybir.dt.bfloat16
Alu = mybir.AluOpType
Act = mybir.ActivationFunctionType

LN2 = math.log(2.0)
# fp32 whose bit pattern is 0x5f3759df (fast-rsqrt magic constant)
MAGIC_F = float(np.int32(0x5F3759DF).view(np.float32))

TRACE = False              # test harness sets this for profiling runs
TRACE_KW = {}
DEBUG = False              # adds intermediate-dump outputs (debugging only)

_cache = {}


class _Bacc(bacc.Bacc):
    """Bacc whose activation-table chooser sees Exp and Ln only in the
    combined natural_log_exp_and_others set, so the log_cosh tail needs one
    table load instead of alternating exp/ln loads."""

    def insert_act_table_loads(self):
        has_activation = any(
            isinstance(i, mybir.InstActivation)
            for b in self.main_func.blocks
            for i in b.instructions
        )
        if not has_activation:
            return
        from concourse.hw_specs import get_activation_tables
        Fn = mybir.ActivationFunctionType
        tables = []
        for name, fns in get_activation_tables(self.m.arch).items():
            if name != "natural_log_exp_and_others":
                fns = fns - {Fn.Exp, Fn.Ln}
            tables.append((name, fns))
        import concourse._compat as _compat  # noqa: F401
        from concourse.bacc import _bass_rust
        _bass_rust.insert_act_table_loads(self, tables)


def _build(cv_nonzero, bf_nonzero, lnf_uniform):
    nc = _Bacc("TRN2", target_bir_lowering=False, debug=False)

    # weights arrive pre-arranged as [partition, ...contiguous] so each
    # DMA is one large descriptor per partition
    xs = nc.dram_tensor("xs", (BPC, N, D), F32, kind="ExternalInput").ap()
    wv = nc.dram_tensor("wv", (128, DC, D), BF, kind="ExternalInput").ap()
    wf = nc.dram_tensor("wf", (128, L, DC, FW), BF, kind="ExternalInput").ap()
    tb_d = nc.dram_tensor("tbank", (128, H, NJ, 128), BF, kind="ExternalInput").ap()
    cv_d = nc.dram_tensor("cv", (D,), F32, kind="ExternalInput").ap()
    bf_d = nc.dram_tensor("bfb", (L, D), F32, kind="ExternalInput").ap()
    lnfs_d = nc.dram_tensor("lnfs", (L, D), F32, kind="ExternalInput").ap()
    lnfb_d = nc.dram_tensor("lnfb", (L, D), F32, kind="ExternalInput").ap()
    out_d = nc.dram_tensor("out", (BPC, N, D), F32, kind="ExternalOutput").ap()

    dbg = None
    if DEBUG:
        dbg = {
            "dbg_u": nc.dram_tensor("dbg_u", (128, AB, D), BF,
                                    kind="ExternalOutput").ap(),
            "dbg_v": nc.dram_tensor("dbg_v", (128, H, NJ, BPC, HS), BF,
                                    kind="ExternalOutput").ap(),
            "dbg_x1": nc.dram_tensor("dbg_x1", (128, BPC, NJ, D), F32,
                                     kind="ExternalOutput").ap(),
            "dbg_y2": nc.dram_tensor("dbg_y2", (128, AB, D), BF,
                                     kind="ExternalOutput").ap(),
            "dbg_z2": nc.dram_tensor("dbg_z2", (128, NJ, D), BF,
                                     kind="ExternalOutput").ap(),
        }

    with tile.TileContext(nc) as tc:
        _emit(nc, tc, xs, wv, wf, tb_d, cv_d, bf_d, lnfs_d, lnfb_d,
              out_d, cv_nonzero, bf_nonzero, lnf_uniform, dbg)
    nc.compile()
    return nc


def _newton1_rstd(nc, pool, dst, var_ap, magict, g):
    """dst[128, g] = 1/sqrt(var + EPS) via bit-hack seed + 1 Newton step
    (max rel err ~0.17%). All tiny DVE ops."""
    vv = pool.tile([128, 16], F32, tag="nvv", name="nvv")[:, 0:g]
    nc.vector.tensor_scalar(vv, var_ap, EPS, None, op0=Alu.add)
    y0 = pool.tile([128, 16], F32, tag="ny0", name="ny0")[:, 0:g]
    nc.vector.tensor_scalar(y0.bitcast(I32), vv.bitcast(I32), 1, None,
                            op0=Alu.logical_shift_right)
    nc.vector.tensor_tensor(y0.bitcast(I32), magict[:, 0:g].bitcast(I32),
                            y0.bitcast(I32), op=Alu.subtract)
    t1 = pool.tile([128, 16], F32, tag="nt1", name="nt1")[:, 0:g]
    nc.vector.tensor_tensor(t1, y0, y0, op=Alu.mult)
    nc.vector.tensor_tensor(t1, t1, vv, op=Alu.mult)
    nc.vector.tensor_scalar(t1, t1, -0.5, 1.5, op0=Alu.mult, op1=Alu.add)
    nc.vector.tensor_tensor(dst, y0, t1, op=Alu.mult)


def _emit(nc, tc, xs, wv, wf, tb_d, cv_d, bf_d, lnfs_d, lnfb_d,
          out_d, cv_nonzero, bf_nonzero, lnf_uniform, dbg=None):
    from contextlib import ExitStack
    ctx = ExitStack()
    with ctx:
        consts = ctx.enter_context(tc.tile_pool(name="consts", bufs=1))
        wp_tb = ctx.enter_context(tc.tile_pool(name="wp_tb", bufs=1))
        wp_wf = ctx.enter_context(tc.tile_pool(name="wp_wf", bufs=1))
        xpool = ctx.enter_context(tc.tile_pool(name="xpool", bufs=1))
        vpool = ctx.enter_context(tc.tile_pool(name="vpool", bufs=1))
        z2p = ctx.enter_context(tc.tile_pool(name="z2p", bufs=2))
        srcp = ctx.enter_context(tc.tile_pool(name="srcp", bufs=3))
        dtp = ctx.enter_context(tc.tile_pool(name="dtp", bufs=3))
        y1p = ctx.enter_context(tc.tile_pool(name="y1p", bufs=2))
        scrp = ctx.enter_context(tc.tile_pool(name="scrp", bufs=2))
        stat = ctx.enter_context(tc.tile_pool(name="stat", bufs=4))
        statp = ctx.enter_context(tc.tile_pool(name="statp", bufs=1))
        ps_mm = ctx.enter_context(tc.tile_pool(name="ps_mm", bufs=4, space="PSUM"))

        # ---- constants ----
        wv_s = consts.tile([128, DC, D], BF, tag="wv")
        # tb/wf DMAs are gated (below) so the 6.6MB of weights don't steal
        # HBM bandwidth from the input-x transfers; the gate tiles alias the
        # weight buffers and are read by a dummy op that depends on a late
        # input chunk's stats.
        tb_gate = wp_tb.tile([128, H, NJ, 128], BF, tag="tb", name="tb_gate")
        wf_gate = wp_wf.tile([128, L, DC, FW], BF, tag="wf", name="wf_gate")
        nc.vector.memset(tb_gate[:, 0, 0, 0:1], 0.0)
        nc.vector.memset(wf_gate[:, 0, 0, 0:1], 0.0)
        magict = consts.tile([128, 16], F32, tag="magic")
        nc.vector.memset(magict[:], MAGIC_F)
        onet = consts.tile([128, 1], F32, tag="one")
        nc.vector.memset(onet[:], 1.0)
        zerot = consts.tile([128, 1], F32, tag="zero")
        nc.vector.memset(zerot[:], 0.0)
        epst = consts.tile([128, 1], F32, tag="eps")
        nc.vector.memset(epst[:], EPS)

        # weight DMAs all go through the gpsimd software queue; tb/wf are
        # emitted mid-phase-A (below) so the input-x transfers get HBM
        # bandwidth first.
        nc.gpsimd.dma_start(wv_s[:], wv)

        cvt = None
        if cv_nonzero:
            cvt = consts.tile([128, D], F32, tag="cv")
            nc.gpsimd.dma_start(cvt[:], cv_d.to_broadcast((128, D)))
        bft = [None] * L
        lnfst = [None] * L
        lnfbt = [None] * L
        for l in range(L):
            if bf_nonzero[l]:
                bft[l] = consts.tile([128, D], F32, tag=f"bf{l}")
                nc.gpsimd.dma_start(bft[l][:], bf_d[l].to_broadcast((128, D)))
            if lnf_uniform[l] is None:
                lnfst[l] = consts.tile([128, D], F32, tag=f"lnfs{l}")
                nc.gpsimd.dma_start(lnfst[l][:], lnfs_d[l].to_broadcast((128, D)))
                lnfbt[l] = consts.tile([128, D], F32, tag=f"lnfb{l}")
                nc.gpsimd.dma_start(lnfbt[l][:], lnfb_d[l].to_broadcast((128, D)))

        # ---- resident tensors ----
        X = xpool.tile([128, BPC, NJ, D], F32, tag="X")         # x, then x1
        V = vpool.tile([128, H, NJ, BPC, HS], BF, tag="V")      # per-head values

        weights = {}
        # ================= phase A: LN + v-projection =================
        # LN's rstd commutes through the matmul: project (mu - x) @ Wv
        # (norm = one ACT Identity with bias = raw mean, nothing else on the
        # pre-matmul critical path), then scale V by -rstd in the post-matmul
        # copy. The reciprocal runs off-path on the DVE.
        mvA = statp.tile([128, NT, 2], F32, tag="mvA")
        nrsA = statp.tile([128, NT], F32, tag="nrsA")
        for g in range(NT // AB):
            b, jc0 = divmod(g * AB, NJ)
            # input DMAs per 2 chunks over three queues: few enough that the
            # DMA semaphore slots don't wrap onto the phase-A transposes
            # (WAR guards), spread for aggregate HBM bandwidth
            engs = (nc.sync, nc.scalar, nc.gpsimd)
            for h2 in range(AB // 2):
                jc = jc0 + h2 * 2
                eng = engs[(g * 2 + h2) % 3]
                eng.dma_start(
                    X[:, b, jc:jc + 2, :],
                    xs[b, jc * 128:(jc + 2) * 128, :].rearrange(
                        "(c p) d -> p c d", p=128))
            ug = srcp.tile([128, AB, D], BF, tag="src", name="ug")
            for ti in range(AB):
                t = g * AB + ti
                xt = X[:, b, jc0 + ti, :]
                st = stat.tile([128, 2, 6], F32, tag="bst")
                nc.vector.bn_stats(st[:, 0, :], xt[:, 0:512])
                nc.vector.bn_stats(st[:, 1, :], xt[:, 512:D])
                nc.vector.bn_aggr(mvA[:, t, :], st[:])
                # u~ = mu - x  (no rstd yet; fires right after this chunk's
                # aggr, independent of any other chunk)
                nc.scalar.activation(ug[:, ti, :], xt, Act.Identity,
                                     bias=mvA[:, t, 0:1], scale=-1.0)
            if g == 2:
                gd = stat.tile([128, 1], F32, tag="gd", name="gd_tb")
                nc.vector.scalar_tensor_tensor(
                    gd[:], mvA[:, g * AB + 3, 0:1], 1.0,
                    tb_gate[:, 0, 0, 0:1], op0=Alu.mult, op1=Alu.add)
                tb_s = wp_tb.tile([128, H, NJ, 128], BF, tag="tb")
                nc.gpsimd.dma_start(tb_s[:], tb_d)
                weights["tb"] = tb_s
            if g == 3:
                gd = stat.tile([128, 1], F32, tag="gd", name="gd_wf")
                nc.vector.scalar_tensor_tensor(
                    gd[:], mvA[:, g * AB + 3, 0:1], 1.0,
                    wf_gate[:, 0, 0, 0:1], op0=Alu.mult, op1=Alu.add)
                wf_s = wp_wf.tile([128, L, DC, FW], BF, tag="wf")
                nc.gpsimd.dma_start(wf_s[:], wf)
                weights["wf"] = wf_s
            gs = slice(g * AB, g * AB + AB)
            # -rstd = reciprocal(-sqrt(var+eps)); off the critical path
            stdt = stat.tile([128, 16], F32, tag="astd", name="astd")[:, 0:AB]
            nc.scalar.activation(stdt, mvA[:, gs, 1], Act.Sqrt, bias=epst[:])
            nstdt = stat.tile([128, 16], F32, tag="anstd", name="anstd")[:, 0:AB]
            nc.scalar.activation(nstdt, stdt, Act.Identity, scale=-1.0)
            nc.vector.reciprocal(nrsA[:, gs], nstdt)
            if dbg is not None and g == 0:
                nc.gpsimd.dma_start(dbg["dbg_u"], ug[:])
            udt = dtp.tile([128, AB * DC, 128], BF, tag="dt", name="udt")
            nc.sync.dma_start_transpose(
                udt[:], ug[:].rearrange("p a d -> p (a d)"))
            for ti in range(AB):
                t = g * AB + ti
                jc = jc0 + ti
                pv = ps_mm.tile([128, 1024], F32, tag="mm")
                for c in range(DC):
                    nc.tensor.matmul(pv[:, 0:512], udt[:, ti * DC + c, :],
                                     wv_s[:, c, 0:512],
                                     start=(c == 0), stop=(c == DC - 1))
                    nc.tensor.matmul(pv[:, 512:D], udt[:, ti * DC + c, :],
                                     wv_s[:, c, 512:D],
                                     start=(c == 0), stop=(c == DC - 1))
                pv3 = pv[:, 0:D].rearrange("p (h k) -> p h k", h=H)
                if cv_nonzero:
                    cv3 = cvt[:].rearrange("p (h k) -> p h k", h=H)
                    nc.vector.scalar_tensor_tensor(V[:, :, jc, b, :], pv3,
                                                   nrsA[:, t:t + 1], cv3,
                                                   op0=Alu.mult, op1=Alu.add)
                else:
                    # V = -rstd * pv = rstd * (x - mu) @ Wv
                    nc.scalar.activation(V[:, :, jc, b, :], pv3, Act.Identity,
                                         scale=nrsA[:, t:t + 1])

        if dbg is not None:
            nc.gpsimd.dma_start(dbg["dbg_v"], V[:])

        # stats tiles for phase C
        ssqC = statp.tile([128, L, NT], F32, tag="ssqC")
        muC = statp.tile([128, L, NT], F32, tag="muC")
        rsC = statp.tile([128, L, NT], F32, tag="rsC")
        biasC = statp.tile([128, L, NT], F32, tag="biasC")
        fence = statp.tile([128, 4], F32, tag="fence")
        mvC = statp.tile([128, AB, 2], F32, tag="mvC")
        inv_d = 1.0 / D

        def phase_b(b):
            # y[ic] = sum_m T[m] @ V[(ic+m) % NJ]; residual into X.
            # two heads share one 2-bank PSUM tile (one accumulation bank
            # each) and get a single fused residual add, halving the DVE op
            # count and the PSUM WAR pressure.
            for h0 in range(0, H, 2):
                pc = ps_mm.tile([128, 2, NJ, HS], F32, tag="mm", name="pc")
                for hh in range(2):
                    h = h0 + hh
                    for m in range(NJ):
                        # one MM per run (free = ln*HS <= 512): a PSUM bank
                        # sees exactly one start=True MM
                        for ic0, jc0, ln in ((0, m, NJ - m), (NJ - m, 0, m)):
                            if ln == 0:
                                continue
                            nc.tensor.matmul(
                                pc[:, hh, ic0:ic0 + ln, :],
                                weights["tb"][:, h, m, :],
                                V[:, h, jc0:jc0 + ln, b, :],
                                start=(m == 0), stop=(m == NJ - 1),
                                skip_group_check=True)
                xap = X[:, b, :, h0 * HS:(h0 + 2) * HS]
                nc.vector.tensor_tensor(
                    xap, xap, pc[:].rearrange("p h j k -> p j h k"),
                    op=Alu.add)

        def c_epi(b, l, g, jc0, o, n, fast, y1g, pffs, y2g, Z2h,
                  var_src=None):
            """stats -> rstd -> Silu for chunk slice [o, o+n) of group g."""
            t0 = b * NJ + jc0 + o
            ts = slice(t0, t0 + n)
            if var_src is None:
                m2 = stat.tile([128, 16], F32, tag="m2", name="m2")[:, 0:n]
                nc.vector.tensor_scalar(m2, ssqC[:, l, ts], inv_d, None,
                                        op0=Alu.mult)
                var = stat.tile([128, 16], F32, tag="var", name="var")[:, 0:n]
                nc.vector.scalar_tensor_tensor(var, muC[:, l, ts], -1.0,
                                               muC[:, l, ts], op0=Alu.mult,
                                               op1=Alu.mult)
                nc.vector.tensor_tensor(var, m2, var, op=Alu.add)
            else:
                var = var_src
            _newton1_rstd(nc, stat, rsC[:, l, ts], var, magict, n)
            if fast:
                cs, cb = lnf_uniform[l]
                if cs != 1.0:
                    nc.vector.tensor_scalar(rsC[:, l, ts], rsC[:, l, ts],
                                            float(cs), None, op0=Alu.mult)
                nc.vector.scalar_tensor_tensor(biasC[:, l, ts], muC[:, l, ts],
                                               -1.0, rsC[:, l, ts],
                                               op0=Alu.mult, op1=Alu.mult)
                if cb != 0.0:
                    nc.vector.tensor_scalar(biasC[:, l, ts], biasC[:, l, ts],
                                            float(cb), None, op0=Alu.add)
                for k in range(n):
                    ti = o + k
                    t = t0 + k
                    ysrc = (pffs[ti][:, 0:D] if pffs[ti] is not None
                            else y1g[:, ti, 0:D])
                    dst = y2g[g][:, ti, :] if l == 0 else Z2h[:, jc0 + ti, :]
                    nc.scalar.activation(dst, ysrc, Act.Silu,
                                         bias=biasC[:, l, t:t + 1],
                                         scale=rsC[:, l, t:t + 1])
            else:
                for k in range(n):
                    ti = o + k
                    t = t0 + k
                    tmp = scrp.tile([128, D], BF, tag="scr", name="lnf_tmp")
                    nc.vector.tensor_scalar(tmp[:], y1g[:, ti, 0:D],
                                            muC[:, l, t:t + 1],
                                            rsC[:, l, t:t + 1],
                                            op0=Alu.subtract, op1=Alu.mult)
                    nc.vector.tensor_tensor(tmp[:], tmp[:], lnfst[l][:],
                                            op=Alu.mult)
                    dst = y2g[g][:, ti, :] if l == 0 else Z2h[:, jc0 + ti, :]
                    nc.vector.tensor_tensor(dst, tmp[:], lnfbt[l][:],
                                            op=Alu.add)
                    nc.scalar.activation(dst, dst, Act.Silu, bias=zerot[:])

        def phase_c(b, tail_cb=None):
            y2g = [None, None]
            xbg = []
            # bf16 casts of x1 up front so they land early in the DVE FIFO
            for g in range(NJ // AB):
                xb = srcp.tile([128, AB, D], BF, tag="src", name="xb")
                nc.vector.tensor_copy(xb[:], X[:, b, g * AB:(g + 1) * AB, :])
                xbg.append(xb)
            Z2h = z2p.tile([128, NJ, D], BF, tag="z2", name=f"z2_{b}")
            for l in range(L):
                fast = lnf_uniform[l] is not None
                # pass 1 (all groups): transposes, matmuls, PSUM-freeing
                # copies, ssq. No Silus/Exps in the ACT FIFO yet, so the
                # PE runs the whole layer without PSUM-release stalls.
                y1gs = []
                for g in range(NJ // AB):
                    jc0 = g * AB
                    src = xbg[g] if l == 0 else y2g[g]
                    if l == 0 and y2g[g] is None:
                        y2g[g] = srcp.tile([128, AB, D], BF, tag="src",
                                           name="y2")
                    y1g = y1p.tile([128, AB, FW], BF, tag="y1")
                    y1gs.append(y1g)
                    # two transpose waves of 2 chunks each
                    for w in range(AB // 2):
                        zdt = dtp.tile([128, 2 * DC, 128], BF, tag="dt",
                                       name="zdt")
                        nc.sync.dma_start_transpose(
                            zdt[:], src[:, w * 2:w * 2 + 2, :].rearrange(
                                "p a d -> p (a d)"))
                        for wi in range(2):
                            ti = w * 2 + wi
                            t = b * NJ + jc0 + ti
                            pff = ps_mm.tile([128, 1024], F32, tag="mm")
                            for c in range(DC):
                                nc.tensor.matmul(pff[:, 0:512],
                                                 zdt[:, wi * DC + c, :],
                                                 weights["wf"][:, l, c, 0:512],
                                                 start=(c == 0),
                                                 stop=(c == DC - 1))
                                nc.tensor.matmul(pff[:, 512:FW],
                                                 zdt[:, wi * DC + c, :],
                                                 weights["wf"][:, l, c, 512:FW],
                                                 start=(c == 0),
                                                 stop=(c == DC - 1))
                            if bf_nonzero[l]:
                                nc.vector.tensor_tensor(pff[:, 0:D],
                                                        pff[:, 0:D],
                                                        bft[l][:], op=Alu.add)
                            # copy 769 cols: dense output + its row-sum
                            nc.scalar.copy(y1g[:, ti, 0:D + 1],
                                           pff[:, 0:D + 1])
                            scr = scrp.tile([128, D], BF, tag="scr")
                            nc.vector.scalar_tensor_tensor(
                                scr[:], y1g[:, ti, 0:D], 0.0,
                                y1g[:, ti, 0:D],
                                op0=Alu.add, op1=Alu.mult,
                                accum_out=ssqC[:, l, t:t + 1])
                # pass 2 (all groups): stats epilogue, Silus, and (for the
                # second layer) the tail's add/abs + optional half-tail
                for g in range(NJ // AB):
                    jc0 = g * AB
                    y1g = y1gs[g]
                    t0 = b * NJ + jc0
                    ts = slice(t0, t0 + AB)
                    nc.vector.tensor_scalar(muC[:, l, ts], y1g[:, :, D],
                                            inv_d, None, op0=Alu.mult)
                    c_epi(b, l, g, jc0, 0, AB, fast, y1g, [None] * AB, y2g,
                          Z2h)
                    if dbg is not None and b == 0 and l == 0 and g == 0:
                        nc.gpsimd.dma_start(dbg["dbg_y2"], y2g[g][:])
                    if l == 1:
                        for ti in range(AB):
                            jc = jc0 + ti
                            xt = X[:, b, jc, :]
                            nc.vector.tensor_tensor(xt, xt, Z2h[:, jc, :],
                                                    op=Alu.add)
                            nc.vector.scalar_tensor_tensor(xt, xt, -1.0, xt,
                                                           op0=Alu.mult,
                                                           op1=Alu.max)
                        if tail_cb is not None:
                            tail_cb(g, Z2h)
            return Z2h

        def tail_part(b, Z2h, jlo, jn, fcell):
            # log_cosh(w) = |w| + log1p(exp(-2|w|)) - ln2; X already holds
            # |w| (add+abs ran inside phase C)
            # fence: depends on the covered layer-2 Silus, used as the Exp
            # bias so tail Exps can't interleave with Silus (table thrash)
            fscr = stat.tile([128, NJ], F32, tag="fscr", name="fscr")[:, 0:jn]
            nc.vector.tensor_scalar(fscr, Z2h[:, jlo:jlo + jn, 0], 0.0, 0.0,
                                    op0=Alu.mult, op1=Alu.mult,
                                    accum_out=fence[:, fcell:fcell + 1])
            for jc in range(jlo, jlo + jn):
                nc.scalar.activation(Z2h[:, jc, :], X[:, b, jc, :], Act.Exp,
                                     bias=fence[:, fcell:fcell + 1],
                                     scale=-2.0)
            for jc in range(jlo, jlo + jn):
                nc.scalar.activation(Z2h[:, jc, :], Z2h[:, jc, :], Act.Ln,
                                     bias=onet[:], scale=1.0)
            for half in range(jn // 2):
                j0 = jlo + half * 2
                for jc in range(j0, j0 + 2):
                    nc.vector.scalar_tensor_tensor(
                        X[:, b, jc, :], Z2h[:, jc, :], -LN2, X[:, b, jc, :],
                        op0=Alu.add, op1=Alu.add)
                nc.sync.dma_start(
                    out_d[b, j0 * 128:(j0 + 2) * 128, :].rearrange(
                        "(c p) d -> p c d", p=128),
                    X[:, b, j0:j0 + 2, :])

        phase_b(0)
        if dbg is not None:
            nc.gpsimd.dma_start(dbg["dbg_x1"][:, 0], X[:, 0])
        Z2_0 = phase_c(0)
        if dbg is not None:
            nc.gpsimd.dma_start(dbg["dbg_z2"], Z2_0[:])
        phase_b(1)
        if dbg is not None:
            nc.gpsimd.dma_start(dbg["dbg_x1"][:, 1], X[:, 1])
        tail_part(0, Z2_0, 0, NJ, 0)
        # batch 1's tail is split per 4-chunk half and emitted inside
        # phase C so the first half overlaps the final matmuls
        Z2_1 = phase_c(1, tail_cb=lambda g, Z: tail_part(1, Z, g * AB, AB,
                                                         2 + g))


def _prep(inputs):
    x = np.asarray(inputs["x"], np.float32)
    ln1_s = np.asarray(inputs["ln1_scale"], np.float32)
    ln1_b = np.asarray(inputs["ln1_bias"], np.float32)
    Wv = np.asarray(inputs["Wv"], np.float32)
    alpha = np.asarray(inputs["alpha"], np.float32)
    Wf = np.asarray(inputs["Wf"], np.float32)
    bfv = np.asarray(inputs["bf"], np.float32)
    lnf_s = np.asarray(inputs["lnf_scale"], np.float32)
    lnf_b = np.asarray(inputs["lnf_bias"], np.float32)

    Wv_flat = Wv.transpose(1, 0, 2).reshape(D, H * HS)
    Wvp = (ln1_s[:, None] * Wv_flat).astype(BF16)
    cv = (ln1_b @ Wv_flat).astype(np.float32)

    # Wf extended with a colsum column (row-sums of the dense output come
    # from the matmul itself) and zero padding to FW
    Wf_ext = np.zeros((L, D, FW), np.float32)
    Wf_ext[:, :, 0:D] = Wf
    Wf_ext[:, :, D] = Wf.sum(axis=2)

    ar = alpha[:, (-np.arange(N)) % N]
    ar2 = np.concatenate([ar, ar], axis=1)
    m_ = np.arange(NJ)[:, None, None]
    p_ = np.arange(128)[None, :, None]
    f_ = np.arange(128)[None, None, :]
    T = ar2[:, N + 128 * m_ + p_ - f_]                  # [H, NJ, 128, 128]
    # [128, H, NJ, 128]: partition-major so the DMA is contiguous
    tbank = np.ascontiguousarray(T.transpose(2, 0, 1, 3)).astype(BF16)

    cv_nonzero = bool(np.any(cv))
    bf_nonzero = tuple(bool(np.any(bfv[l])) for l in range(L))
    lnf_uniform = []
    for l in range(L):
        s, bb = lnf_s[l], lnf_b[l]
        if np.all(s == s[0]) and np.all(bb == bb[0]):
            lnf_uniform.append((float(s[0]), float(bb[0])))
        else:
            lnf_uniform.append(None)
    key = (cv_nonzero, bf_nonzero, tuple(lnf_uniform))

    # partition-major weight layouts for contiguous DMA:
    # wv: [D, H*HS] -> [128, DC, D_out];  wf: [L, D, FW] -> [128, L, DC, FW]
    wv_pm = np.ascontiguousarray(
        Wvp.reshape(DC, 128, D).transpose(1, 0, 2))
    wf_pm = np.ascontiguousarray(
        Wf_ext.astype(BF16).reshape(L, DC, 128, FW).transpose(2, 0, 1, 3))
    common = {
        "wv": wv_pm,
        "wf": wf_pm,
        "tbank": tbank,
        "cv": cv,
        "bfb": bfv,
        "lnfs": lnf_s,
        "lnfb": lnf_b,
    }
    return x, key, common, (cv_nonzero, bf_nonzero, lnf_uniform)


def kernel(**inputs):
    x, key, common, flags = _prep(inputs)
    if key not in _cache:
        _cache[key] = _build(*flags)
    nc = _cache[key]
    in_maps = []
    for i in range(NCORES):
        m = dict(common)
        m["xs"] = np.ascontiguousarray(x[i * BPC:(i + 1) * BPC])
        in_maps.append(m)
    res = run_bass_kernel_spmd(nc, in_maps, core_ids=list(range(NCORES)),
                               trace=TRACE, **TRACE_KW)
    kernel.last_result = res
    out = np.empty((B, N, D), np.float32)
    for i in range(NCORES):
        out[i * BPC:(i + 1) * BPC] = res.results[i]["out"]
    return out
